# revision 1
# baseline (speedup 1.0000x reference)
"""Single-head attention (B=2, S=2048, D=2048, fp32) on 8 trn2 NeuronCores.

Sharding: sequence-parallel. The 4096 tokens (B*S) are split 512/core; cores
0-3 hold batch 0, cores 4-7 batch 1. Each core computes Q^T, K^T, V for its
512 tokens, K^T/V shards are all-gathered within each 4-core group (one group
per batch), then each core computes scores -> softmax -> attn@V -> @W_o for
its 512 queries against the full 2048 keys of its batch.

All matmuls run as fp32r (full PE rate at N=512, ~tf32 precision; inputs and
all producer chains typed float32r to satisfy the BIR verifier).

per-core phases (each 256 matmuls of K=128, M=128, N=512):
  B: KT_shard(e,t)  = mm(lhsT=W_k[d,e], rhs=xt[d,t])      -> DRAM, AllGather
  C: V_shard(t,e)   = mm(lhsT=xt[d,t],  rhs=W_v[d,e])     -> DRAM, AllGather
  D: QT(e,q)        = mm(lhsT=W_q'[d,e], rhs=xt[d,q])     -> SBUF  (W_q'=W_q/sqrt(D))
  E: scores(q,k)    = mm(lhsT=QT[e,q], rhs=KTg[e,k]); exp (no max-sub; |s|<~8)
     with accum_out row-sums; attnT via PE transposes
  F: outT(e,q)      = mm(lhsT=Vg[k,e], rhs=attnT[k,q])
  G: final(q,d)     = mm(lhsT=outT[e,q], rhs=W_o[e,d]) * (1/rowsum) -> out DRAM

All DMA loads are (128, 512) chunks (2KB per partition line).
"""
import math
import numpy as np

import concourse.bass as bass
import concourse.mybir as mybir
import concourse.tile as tile
from concourse import bacc
from concourse.bass_utils import run_bass_kernel_spmd
from concourse.masks import make_identity

F32 = mybir.dt.float32
F32R = mybir.dt.float32r

D = 2048          # d_model
B = 2
S = 2048
NCORES = 8
GS = 4            # group size (cores per batch)
TOK = 512         # tokens per core
P = 128
NT = D // P       # 16 tiles along d/e
QT_N = TOK // P   # 4 q tiles per core
KC_N = S // 512   # 4 key chunks of 512


def build_attn(n_iters=1, with_collective=True, psum_bufs=8, w_bufs=24, kv_bufs=8, skip_w_reload=False):
    """Build the SPMD attention kernel. n_iters>1 wraps ALL compute phases in
    a timing loop (collectives must be disabled for that)."""
    assert n_iters == 1 or with_collective is False
    nc = bacc.Bacc("TRN2", target_bir_lowering=False, debug=False, num_devices=NCORES)

    xt = nc.dram_tensor("xt", [D, TOK], F32R, kind="ExternalInput")
    wq = nc.dram_tensor("wq", [D, D], F32R, kind="ExternalInput")
    wk = nc.dram_tensor("wk", [D, D], F32R, kind="ExternalInput")
    wv = nc.dram_tensor("wv", [D, D], F32R, kind="ExternalInput")
    wo = nc.dram_tensor("wo", [D, D], F32R, kind="ExternalInput")
    out = nc.dram_tensor("out", [TOK, D], F32, kind="ExternalOutput")

    with tile.TileContext(nc) as tc:
        with (
            tc.tile_pool(name="dram", bufs=1, space="DRAM") as dram,
            tc.tile_pool(name="big", bufs=1) as big,
            tc.tile_pool(name="stream", bufs=w_bufs) as stream,
            tc.tile_pool(name="qtpool", bufs=NT) as qtpool,
            tc.tile_pool(name="evpool", bufs=3) as evpool,
            tc.tile_pool(name="attnpool", bufs=4) as attnpool,
            tc.tile_pool(name="misc", bufs=1) as misc,
            tc.tile_pool(name="ps", bufs=psum_bufs, space="PSUM") as ps,
        ):
            kt_shard = dram.tile([D, TOK], F32R)
            v_shard = dram.tile([TOK, D], F32R)
            kt_g = dram.tile([GS * D, TOK], F32R)    # [s*D + e, k_local]
            v_g = dram.tile([GS * TOK, D], F32R)     # [k, e]

            xt_sb = [big.tile([P, TOK], F32R, name=f"xt{i}") for i in range(NT)]

            attnT_sb = [big.tile([P, TOK], F32R, name=f"attnT{i}") for i in range(NT)]


            ident = misc.tile([P, P], F32)
            make_identity(nc, ident)
            sums = misc.tile([P, QT_N, KC_N], F32)
            recip = misc.tile([P, QT_N], F32)
            tot = misc.tile([P, QT_N], F32)

            _eng_i = [0]
            _engines = [nc.sync, nc.scalar]

            def LD(dst, src_ap):
                e = _engines[_eng_i[0] % len(_engines)]
                _eng_i[0] += 1
                e.dma_start(dst, src_ap)

            # ---- load x^T into SBUF
            for dt in range(NT):
                nc.sync.dma_start(xt_sb[dt][:], xt[dt * P:(dt + 1) * P, :])

            def stream_group(pfx, load_fn):
                """Load 16 (128,512) chunks via round-robin engines."""
                ts = []
                for i in range(NT):
                    t = stream.tile([P, 512], F32R, tag="stream", name=f"{pfx}{i}")
                    if i == 0 or not skip_w_reload:
                        LD(t[:], load_fn(i))
                        ts.append(t)
                    else:
                        ts.append(ts[0])
                return ts

            def proj_to_T(w_dram, dest_cb, pfx):
                """QT/KT-style projection: out[e,t] = sum_d W[d,e]*xt[d,t].
                16-deep same-bank accumulation chains (chain16 pattern)."""
                for eg in range(NT // 4):
                    wts = stream_group(pfx, lambda dt: w_dram[
                        dt * P:(dt + 1) * P, eg * 512:(eg + 1) * 512])
                    psums = [ps.tile([P, 512], F32, tag="mm", name=f"{pfx}p{i}")
                             for i in range(4)]
                    for half in range(2):
                        for j in range(4):
                            for dt8 in range(8):
                                dt = half * 8 + dt8
                                nc.tensor.matmul(
                                    psums[j][:], wts[dt][:, j * P:(j + 1) * P],
                                    xt_sb[dt][:],
                                    start=(dt == 0), stop=(dt == NT - 1))
                    for j in range(4):
                        dest_cb(eg * 4 + j, psums[j])

            def b_dest(et, psum):
                ev = evpool.tile([P, 512], F32R, tag="ev", name="evb")
                nc.scalar.copy(ev[:], psum[:])
                nc.sync.dma_start(kt_shard[et * P:(et + 1) * P, :], ev[:])

            def phase_c():
                for ec in range(4):
                    wvs = stream_group("cw", lambda dt: wv[
                        dt * P:(dt + 1) * P, ec * 512:(ec + 1) * 512])
                    psums = [ps.tile([P, 512], F32, tag="mm", name=f"pvp{i}")
                             for i in range(4)]
                    for half in range(2):
                        for tt in range(QT_N):
                            for dt8 in range(8):
                                dt = half * 8 + dt8
                                nc.tensor.matmul(
                                    psums[tt][:], xt_sb[dt][:, tt * P:(tt + 1) * P],
                                    wvs[dt][:],
                                    start=(dt == 0), stop=(dt == NT - 1))
                    for tt in range(QT_N):
                        ev = evpool.tile([P, 512], F32R, tag="ev", name="evc")
                        nc.scalar.copy(ev[:], psums[tt][:])
                        nc.sync.dma_start(
                            v_shard[tt * P:(tt + 1) * P, ec * 512:(ec + 1) * 512], ev[:])

            def phases_defg():
                # ---- phase D: QT (tiles share slots with outT via tag)
                qt_sb = [qtpool.tile([P, TOK], F32R, tag="qo", name=f"qt{i}")
                         for i in range(NT)]

                def d_dest(et, psum):
                    nc.scalar.copy(qt_sb[et][:], psum[:])
                proj_to_T(wq, d_dest, "pd")

                # ---- phase E: scores + exp + inline transposes
                for kc in range(KC_N):
                    kts = stream_group("ek", lambda et: kt_g[
                        kc * D + et * P: kc * D + (et + 1) * P, :])
                    psums = [ps.tile([P, 512], F32, tag="mm", name=f"pep{i}")
                             for i in range(4)]
                    for half in range(2):
                        for qt in range(QT_N):
                            for et8 in range(8):
                                et = half * 8 + et8
                                nc.tensor.matmul(
                                    psums[qt][:],
                                    qt_sb[et][:, qt * P:(qt + 1) * P],
                                    kts[et][:],
                                    start=(et == 0), stop=(et == NT - 1))
                    for qt in range(QT_N):
                        ax = attnpool.tile([P, 512], F32, tag="ax", name="ax")
                        nc.scalar.activation(
                            ax[:], psums[qt][:],
                            mybir.ActivationFunctionType.Exp,
                            accum_out=sums[:, qt, kc:kc + 1])
                        for j in range(4):
                            pt = ps.tile([P, P], F32, tag="mm", name="pt")
                            nc.tensor.transpose(
                                pt[:], ax[:, j * P:(j + 1) * P], ident[:])
                            nc.scalar.copy(
                                attnT_sb[kc * 4 + j][:, qt * P:(qt + 1) * P], pt[:])
                # row sums + reciprocal
                for qt in range(QT_N):
                    nc.vector.reduce_sum(tot[:, qt:qt + 1], sums[:, qt, :],
                                         axis=mybir.AxisListType.X)
                nc.vector.reciprocal(recip[:], tot[:])

                # ---- phase F: outT (slots freed by qt after phase E)
                outT_sb = [qtpool.tile([P, TOK], F32R, tag="qo", name=f"outT{i}")
                           for i in range(NT)]
                for eg in range(NT // 4):
                    vts = stream_group("fv", lambda kt: v_g[
                        kt * P:(kt + 1) * P, eg * 512:(eg + 1) * 512])
                    psums = [ps.tile([P, 512], F32, tag="mm", name=f"pfp{i}")
                             for i in range(4)]
                    for half in range(2):
                        for j in range(4):
                            for kt8 in range(8):
                                kt = half * 8 + kt8
                                nc.tensor.matmul(
                                    psums[j][:], vts[kt][:, j * P:(j + 1) * P],
                                    attnT_sb[kt][:],
                                    start=(kt == 0), stop=(kt == NT - 1))
                    for j in range(4):
                        nc.scalar.copy(outT_sb[eg * 4 + j][:], psums[j][:])

                # ---- phase G: final
                for dc in range(4):
                    wos = stream_group("gw", lambda et: wo[
                        et * P:(et + 1) * P, dc * 512:(dc + 1) * 512])
                    psums = [ps.tile([P, 512], F32, tag="mm", name=f"pgp{i}")
                             for i in range(4)]
                    for half in range(2):
                        for qt in range(QT_N):
                            for et8 in range(8):
                                et = half * 8 + et8
                                nc.tensor.matmul(
                                    psums[qt][:],
                                    outT_sb[et][:, qt * P:(qt + 1) * P],
                                    wos[et][:],
                                    start=(et == 0), stop=(et == NT - 1))
                    for qt in range(QT_N):
                        evf = evpool.tile([P, 512], F32, tag="evf")
                        nc.vector.tensor_scalar_mul(evf[:], psums[qt][:],
                                                    recip[:, qt:qt + 1])
                        nc.sync.dma_start(
                            out[qt * P:(qt + 1) * P, dc * 512:(dc + 1) * 512], evf[:])

            def whole_body():
                proj_to_T(wk, b_dest, "pb")
                if with_collective in (True, "k"):
                    nc.gpsimd.collective_compute(
                        "AllGather", mybir.AluOpType.bypass,
                        replica_groups=[[0, 1, 2, 3], [4, 5, 6, 7]],
                        ins=[kt_shard[:].opt()], outs=[kt_g[:].opt()],
                    )
                phase_c()
                if with_collective in (True, "v"):
                    nc.gpsimd.collective_compute(
                        "AllGather", mybir.AluOpType.bypass,
                        replica_groups=[[0, 1, 2, 3], [4, 5, 6, 7]],
                        ins=[v_shard[:].opt()], outs=[v_g[:].opt()],
                    )
                phases_defg()

            if n_iters == 1:
                whole_body()
            else:
                with tc.For_i(0, n_iters, 1):
                    whole_body()

    nc.compile()
    return nc


_CACHED = {}


def _get_nc():
    if "nc" not in _CACHED:
        _CACHED["nc"] = build_attn()
    return _CACHED["nc"]


def _make_in_maps(inputs):
    x = np.asarray(inputs["x"], np.float32)
    W_q = np.asarray(inputs["W_q"], np.float32)
    W_k = np.asarray(inputs["W_k"], np.float32)
    W_v = np.asarray(inputs["W_v"], np.float32)
    W_o = np.asarray(inputs["W_o"], np.float32)

    scale = np.float32(1.0 / math.sqrt(D))
    wq_s = np.ascontiguousarray(W_q * scale)
    wk_c = np.ascontiguousarray(W_k)
    wv_c = np.ascontiguousarray(W_v)
    wo_c = np.ascontiguousarray(W_o)

    toks = x.reshape(B * S, D)              # (4096, 2048)
    xt_full = np.ascontiguousarray(toks.T)  # (2048, 4096)

    in_maps = []
    for c in range(NCORES):
        in_maps.append({
            "xt": np.ascontiguousarray(xt_full[:, c * TOK:(c + 1) * TOK]),
            "wq": wq_s, "wk": wk_c, "wv": wv_c, "wo": wo_c,
        })
    return in_maps


def kernel(x, W_q, W_k, W_v, W_o):
    in_maps = _make_in_maps(dict(x=x, W_q=W_q, W_k=W_k, W_v=W_v, W_o=W_o))
    nc = _get_nc()
    res = run_bass_kernel_spmd(nc, in_maps, core_ids=list(range(NCORES)))
    rows = np.concatenate([res.results[c]["out"] for c in range(NCORES)], axis=0)
    return rows.reshape(B, S, D)



# revision 9
# speedup vs baseline: 1.1440x; 1.1440x over previous
"""Single-head attention (B=2, S=2048, D=2048, fp32 in/out) on 8 trn2 NeuronCores.

Sharding: sequence-parallel. The 4096 tokens (B*S) are split 512/core; cores
0-3 hold batch 0, cores 4-7 batch 1. Each core computes Q^T, K^T, V for its
512 tokens, K^T/V shards are all-gathered within each 4-core group (one group
per batch), then each core computes scoresT -> exp -> attn@V -> @W_o for
its 512 queries against the full 2048 keys of its batch.

All matmuls run in bf16 (1 cycle/row at N=512, fp32 PSUM accumulation).
Host converts x and weights to bf16; K^T/V shards are produced in bf16 so
the AllGathers move half the bytes of fp32.

per-core phases (each 256 matmuls of K=128, M=128, N=512):
  B: KT_shard(e,t)  = mm(lhsT=W_k[d,e], rhs=xt[d,t])      -> DRAM, AllGather
  C: V_shard(t,e)   = mm(lhsT=xt[d,t],  rhs=W_v[d,e])     -> DRAM, AllGather
  D: QT(e,q)        = mm(lhsT=W_q'[d,e], rhs=xt[d,q])     -> SBUF  (W_q'=W_q/sqrt(D))
  E: scoresT(k,q)   = mm(lhsT=KTg[e,k], rhs=QT[e,q]); exp -> attnT (bf16, direct;
     no PE transposes). rowsums via 16 mms lhsT=ones[128,128], rhs=attnT tile
     -> psum[128,512] broadcast on all partitions; reciprocal on DVE.
  F: outT(e,q)      = mm(lhsT=Vg[k,e], rhs=attnT[k,q]); evacuation fused with
     normalization: outT_sb = psum * recip_bcast (DVE)
  G: final(q,d)     = mm(lhsT=outT[e,q], rhs=W_o[e,d]) -> out DRAM (fp32)

DMA-instruction count is minimized (HWDGE costs ~630ns fixed per dma_start):
weight/V streams load [128,1024] bf16 chunks (2KB lines); C/G outputs are
batched into [128,2048] row tiles and stored with 4 DMAs per phase.
"""
import math
import numpy as np
import ml_dtypes

import concourse.bass as bass
import concourse.mybir as mybir
import concourse.tile as tile
from concourse import bacc
from concourse.bass_utils import run_bass_kernel_spmd

F32 = mybir.dt.float32
BF16 = mybir.dt.bfloat16

D = 2048          # d_model
B = 2
S = 2048
NCORES = 8
GS = 4            # group size (cores per batch)
TOK = 512         # tokens per core
P = 128
NT = D // P       # 16 tiles along d/e
QT_N = TOK // P   # 4 q tiles per core
KC_N = S // 512   # 4 key chunks of 512


def build_attn(n_iters=1, with_collective=True, psum_bufs=7, w_bufs=None, k_bufs=24,
               chunk=1024):
    NSUB = chunk // 512
    NSUP = D // chunk
    if w_bufs is None:
        w_bufs = 24 * 1024 // chunk
    """Build the SPMD attention kernel. n_iters>1 wraps ALL compute phases in
    a timing loop (collectives must be disabled for that)."""
    assert n_iters == 1 or with_collective is False
    nc = bacc.Bacc("TRN2", target_bir_lowering=False, debug=False, num_devices=NCORES)

    xt = nc.dram_tensor("xt", [D, TOK], BF16, kind="ExternalInput")
    wq = nc.dram_tensor("wq", [D, D], BF16, kind="ExternalInput")
    wk = nc.dram_tensor("wk", [D, D], BF16, kind="ExternalInput")
    wv = nc.dram_tensor("wv", [D, D], BF16, kind="ExternalInput")
    wo = nc.dram_tensor("wo", [D, D], BF16, kind="ExternalInput")
    out = nc.dram_tensor("out", [TOK, D], F32, kind="ExternalOutput")

    with tile.TileContext(nc) as tc:
        with (
            tc.tile_pool(name="dram", bufs=1, space="DRAM") as dram,
            tc.tile_pool(name="big", bufs=1) as big,
            tc.tile_pool(name="wide", bufs=w_bufs) as wide,
            tc.tile_pool(name="kpool", bufs=k_bufs) as kpool,
            tc.tile_pool(name="qtpool", bufs=NT) as qtpool,
            tc.tile_pool(name="rows", bufs=4) as rows,
            tc.tile_pool(name="evpool", bufs=4) as evpool,
            tc.tile_pool(name="misc", bufs=1) as misc,
            tc.tile_pool(name="ps", bufs=psum_bufs, space="PSUM") as ps,
            tc.tile_pool(name="ps_rs", bufs=1, space="PSUM") as ps_rs,
        ):
            kt_shard = dram.tile([D, TOK], BF16)
            v_shard = dram.tile([TOK, D], BF16)
            kt_g = dram.tile([GS * D, TOK], BF16)    # [s*D + e, k_local]
            v_g = dram.tile([GS * TOK, D], BF16)     # [k, e]

            xt_sb = [big.tile([P, TOK], BF16, name=f"xt{i}") for i in range(NT)]
            attnT_sb = [big.tile([P, TOK], BF16, name=f"attnT{i}") for i in range(NT)]

            ones_sb = misc.tile([P, P], BF16)
            nc.gpsimd.memset(ones_sb[:], 1.0)
            recip_bcast = misc.tile([P, TOK], F32)

            _eng_i = [0]
            _engines = [nc.sync, nc.scalar]

            def LD(dst, src_ap, eng=None):
                e = eng if eng is not None else _engines[_eng_i[0] % len(_engines)]
                _eng_i[0] += 1
                e.dma_start(dst, src_ap)

            # ---- load x^T into SBUF (sync queue; phase B's first weight
            # group goes on scalar so PE can start after ~2 tiles land)
            for dt in range(NT):
                nc.sync.dma_start(xt_sb[dt][:], xt[dt * P:(dt + 1) * P, :])

            def wide_group(pfx, load_fn, eng=None):
                """Load 16 (128,chunk) chunks via round-robin engines."""
                ts = []
                for i in range(NT):
                    t = wide.tile([P, chunk], BF16, tag="wide", name=f"{pfx}{i}")
                    LD(t[:], load_fn(i), eng)
                    ts.append(t)
                return ts

            def proj_to_T(w_dram, dest_cb, pfx, first_eng=None):
                """QT/KT-style projection: out[e,t] = sum_d W[d,e]*xt[d,t].
                16-deep same-bank accumulation chains, [128,1024] loads."""
                for eg2 in range(NSUP):
                    wts = wide_group(pfx, lambda dt: w_dram[
                        dt * P:(dt + 1) * P, eg2 * chunk:(eg2 + 1) * chunk],
                        eng=first_eng if eg2 == 0 else None)
                    for sub in range(NSUB):
                        eg = eg2 * NSUB + sub
                        psums = [ps.tile([P, 512], F32, tag="mm", name=f"{pfx}p{i}")
                                 for i in range(4)]
                        for half in range(2):
                            for j in range(4):
                                for dt8 in range(8):
                                    dt = half * 8 + dt8
                                    nc.tensor.matmul(
                                        psums[j][:],
                                        wts[dt][:, sub * 512 + j * P:
                                                sub * 512 + (j + 1) * P],
                                        xt_sb[dt][:],
                                        start=(dt == 0), stop=(dt == NT - 1))
                        for j in range(4):
                            dest_cb(eg * 4 + j, psums[j])

            def b_dest(et, psum):
                ev = evpool.tile([P, 512], BF16, tag="ev", name="evb")
                nc.scalar.copy(ev[:], psum[:])
                nc.sync.dma_start(kt_shard[et * P:(et + 1) * P, :], ev[:])

            def phase_c():
                v_rows = [rows.tile([P, D], BF16, tag="vr", name=f"vr{i}")
                          for i in range(QT_N)]
                for ec2 in range(NSUP):
                    wvs = wide_group("cw", lambda dt: wv[
                        dt * P:(dt + 1) * P, ec2 * chunk:(ec2 + 1) * chunk])
                    for sub in range(NSUB):
                        ec = ec2 * NSUB + sub
                        psums = [ps.tile([P, 512], F32, tag="mm", name=f"pvp{i}")
                                 for i in range(4)]
                        for half in range(2):
                            for tt in range(QT_N):
                                for dt8 in range(8):
                                    dt = half * 8 + dt8
                                    nc.tensor.matmul(
                                        psums[tt][:],
                                        xt_sb[dt][:, tt * P:(tt + 1) * P],
                                        wvs[dt][:, sub * 512:(sub + 1) * 512],
                                        start=(dt == 0), stop=(dt == NT - 1))
                        for tt in range(QT_N):
                            nc.scalar.copy(
                                v_rows[tt][:, ec * 512:(ec + 1) * 512],
                                psums[tt][:])
                for tt in range(QT_N):
                    nc.sync.dma_start(
                        v_shard[tt * P:(tt + 1) * P, :], v_rows[tt][:])

            def phases_defg():
                # ---- phase D: QT (tiles share slots with outT via tag)
                qt_sb = [qtpool.tile([P, TOK], BF16, tag="qo", name=f"qt{i}")
                         for i in range(NT)]

                def d_dest(et, psum):
                    nc.scalar.copy(qt_sb[et][:], psum[:])
                proj_to_T(wq, d_dest, "pd")

                # ---- phase E: scoresT -> exp -> attnT (no transposes)
                for kc in range(KC_N):
                    kts = []
                    for et in range(NT):
                        t = kpool.tile([P, 512], BF16, tag="kt", name=f"ek{et}")
                        LD(t[:], kt_g[kc * D + et * P: kc * D + (et + 1) * P, :])
                        kts.append(t)
                    psums = [ps.tile([P, 512], F32, tag="mm", name=f"pep{i}")
                             for i in range(4)]
                    for half in range(2):
                        for j in range(4):
                            for et8 in range(8):
                                et = half * 8 + et8
                                nc.tensor.matmul(
                                    psums[j][:],
                                    kts[et][:, j * P:(j + 1) * P],
                                    qt_sb[et][:],
                                    start=(et == 0), stop=(et == NT - 1))
                    for j in range(4):
                        nc.scalar.activation(
                            attnT_sb[kc * 4 + j][:], psums[j][:],
                            mybir.ActivationFunctionType.Exp)

                # rowsums broadcast to all partitions: rs[p,q] = sum_k attnT[k,q]
                rs_ps = ps_rs.tile([P, TOK], F32, name="rs_ps")
                for t in range(NT):
                    nc.tensor.matmul(
                        rs_ps[:], ones_sb[:], attnT_sb[t][:],
                        start=(t == 0), stop=(t == NT - 1))
                nc.vector.reciprocal(recip_bcast[:], rs_ps[:])

                # ---- phase F: outT (slots freed by qt after phase E)
                outT_sb = [qtpool.tile([P, TOK], BF16, tag="qo", name=f"outT{i}")
                           for i in range(NT)]
                for eg2 in range(NSUP):
                    vts = wide_group("fv", lambda kt: v_g[
                        kt * P:(kt + 1) * P, eg2 * chunk:(eg2 + 1) * chunk])
                    for sub in range(NSUB):
                        eg = eg2 * NSUB + sub
                        psums = [ps.tile([P, 512], F32, tag="mm", name=f"pfp{i}")
                                 for i in range(4)]
                        for half in range(2):
                            for j in range(4):
                                for kt8 in range(8):
                                    kt = half * 8 + kt8
                                    nc.tensor.matmul(
                                        psums[j][:],
                                        vts[kt][:, sub * 512 + j * P:
                                                sub * 512 + (j + 1) * P],
                                        attnT_sb[kt][:],
                                        start=(kt == 0), stop=(kt == NT - 1))
                        for j in range(4):
                            nc.vector.tensor_mul(
                                outT_sb[eg * 4 + j][:], psums[j][:],
                                recip_bcast[:])

                # ---- phase G: final
                out_rows = [rows.tile([P, D], F32, tag="or", name=f"orow{i}")
                            for i in range(QT_N)]
                for dc2 in range(NSUP):
                    wos = wide_group("gw", lambda et: wo[
                        et * P:(et + 1) * P, dc2 * chunk:(dc2 + 1) * chunk])
                    for sub in range(NSUB):
                        dc = dc2 * NSUB + sub
                        psums = [ps.tile([P, 512], F32, tag="mm", name=f"pgp{i}")
                                 for i in range(4)]
                        for half in range(2):
                            for qt in range(QT_N):
                                for et8 in range(8):
                                    et = half * 8 + et8
                                    nc.tensor.matmul(
                                        psums[qt][:],
                                        outT_sb[et][:, qt * P:(qt + 1) * P],
                                        wos[et][:, sub * 512:(sub + 1) * 512],
                                        start=(et == 0), stop=(et == NT - 1))
                        for qt in range(QT_N):
                            nc.scalar.copy(
                                out_rows[qt][:, dc * 512:(dc + 1) * 512],
                                psums[qt][:])
                for qt in range(QT_N):
                    nc.sync.dma_start(out[qt * P:(qt + 1) * P, :], out_rows[qt][:])

            def whole_body(first=False):
                proj_to_T(wk, b_dest, "pb",
                          first_eng=nc.scalar if first else None)
                if with_collective in (True, "k"):
                    nc.gpsimd.collective_compute(
                        "AllGather", mybir.AluOpType.bypass,
                        replica_groups=[[0, 1, 2, 3], [4, 5, 6, 7]],
                        ins=[kt_shard[:].opt()], outs=[kt_g[:].opt()],
                    )
                phase_c()
                if with_collective in (True, "v"):
                    nc.gpsimd.collective_compute(
                        "AllGather", mybir.AluOpType.bypass,
                        replica_groups=[[0, 1, 2, 3], [4, 5, 6, 7]],
                        ins=[v_shard[:].opt()], outs=[v_g[:].opt()],
                    )
                phases_defg()

            if n_iters == 1:
                whole_body(first=True)
            else:
                with tc.For_i(0, n_iters, 1):
                    whole_body()

    nc.compile()
    return nc


_CACHED = {}


def _get_nc():
    if "nc" not in _CACHED:
        _CACHED["nc"] = build_attn()
    return _CACHED["nc"]


def _make_in_maps(inputs):
    x = np.asarray(inputs["x"], np.float32)
    W_q = np.asarray(inputs["W_q"], np.float32)
    W_k = np.asarray(inputs["W_k"], np.float32)
    W_v = np.asarray(inputs["W_v"], np.float32)
    W_o = np.asarray(inputs["W_o"], np.float32)

    bf = ml_dtypes.bfloat16
    scale = np.float32(1.0 / math.sqrt(D))
    wq_s = np.ascontiguousarray((W_q * scale).astype(bf))
    wk_c = np.ascontiguousarray(W_k.astype(bf))
    wv_c = np.ascontiguousarray(W_v.astype(bf))
    wo_c = np.ascontiguousarray(W_o.astype(bf))

    toks = x.reshape(B * S, D)              # (4096, 2048)
    xt_full = np.ascontiguousarray(toks.T.astype(bf))  # (2048, 4096)

    in_maps = []
    for c in range(NCORES):
        in_maps.append({
            "xt": np.ascontiguousarray(xt_full[:, c * TOK:(c + 1) * TOK]),
            "wq": wq_s, "wk": wk_c, "wv": wv_c, "wo": wo_c,
        })
    return in_maps


def kernel(x, W_q, W_k, W_v, W_o):
    in_maps = _make_in_maps(dict(x=x, W_q=W_q, W_k=W_k, W_v=W_v, W_o=W_o))
    nc = _get_nc()
    res = run_bass_kernel_spmd(nc, in_maps, core_ids=list(range(NCORES)))
    rows = np.concatenate([res.results[c]["out"] for c in range(NCORES)], axis=0)
    return rows.reshape(B, S, D)
